# revision 1
# baseline (speedup 1.0000x reference)
"""Trainium2 Bass kernel for batched CCNeuron simulation.

Reference semantics (per neuron b, per step t):
    pv  = 0.75*pv + 0.25*relu(W_pv@x + y*w_pv_lat + noise_p)
    a   = 0.98*a  + 0.02*y
    y   = 0.9*y   + 0.1*relu(w_ff.x + w_fb.(c*rc) - w_lat.pv_new + ny - a_new)
    out = (y_old, y_new, pv_new)

Sharding: pure data parallel on B across 8 cores (4096 neurons/core).
Per-core layout: partition p = b%128, free lane j = b//128 (J=32 lanes).

Device algorithm per core:
  - bulk (GPSIMD, per chunk of K steps, written in-place into the input
    tile): G[t] = W_pv@x + noise_p (2/neuron), H[t] = w_ff.x + w_fb_eff.c
    + ny (1/neuron)
  - sequential scan (DVE, per step), states A=a/0.02, L=w_lat.pv, Y, PV:
      A   = 0.98*A + Y_prev
      Pt  = H - 0.02*A
      s   = G + Y_prev*w_pv_lat
      qs  = max(s,0)*0.25
      PV  = 0.75*PV_prev + qs          -> pv history (output)
      u1  = pairsum(qs*w_lat)
      L   = 0.75*L + u1                ( == w_lat.pv_new exactly )
      arg = Pt - L
      r   = max(0.1*arg, 0)
      Y   = 0.9*Y_prev + r             -> y history (output)
y_prev output column is reconstructed on host as shift(y_next) with y0.

Hardware constraint that shapes the structure: every compute/DMA
instruction fits only ONE inline sync-wait, and there are 8 SWDGE + 8
HWDGE completion-sem lanes. So: exactly 8 input DMAs (one per 64-step
group, weights+init packed in front of group 0) and 8 output DMAs (one
per group), with small observer/absorber ops arranged so no instruction
ever needs two fresh semaphore waits.
"""

import numpy as np

T, B, F, P, C = 512, 32768, 2, 2, 2
NCORES = 8
BS = B // NCORES      # 4096 neurons per core
PPART = 128           # SBUF partitions
J = BS // PPART       # 32 free lanes per partition
K = 32                # timesteps per chunk
NCHUNK = T // K
KG = 2 * K            # timesteps per input/output group (8 groups)
WPL = 16 * J          # weights+state prefix floats (12 wt planes + 4 s0)

_PROGRAM_CACHE = {}


def _patch_drain_split():
    """The kernel-tail drain carries one wait per live semaphore lane; with
    8 SWDGE + 8 HWDGE lanes in use it overflows the instruction's sync-wait
    capacity. Split the waits over several drain instructions."""
    import concourse.tile as tile_mod
    from concourse.vector_clock import ScopedClock, VectorClock

    if getattr(tile_mod.TileContext, "_drain_split_patched", False):
        return

    def _drain_and_barrier(self, tick_clock, wait_clock):
        gc = tick_clock.global_clock
        n = len(gc)
        idxs = [i for i in range(n) if gc[i] > 0]
        for s in range(0, len(idxs), 1):
            grp = set(idxs[s:s + 1])
            vc = VectorClock([gc[i] if i in grp else 0 for i in range(n)])
            di = self.nc.sync.drain()
            wait_clock.add_sem_waits(di.ins, ScopedClock({None: vc}))
        if not idxs:
            di = self.nc.sync.drain()
            wait_clock.add_sem_waits(
                di.ins, ScopedClock({None: tick_clock.global_clock})
            )
        self.nc.all_engine_barrier()
        assert self.sems is not None
        popped = self.nc._tile_sem_poison_stack.pop()
        assert popped is self._sem_poison
        self.nc.clear_and_free_semaphores(list(self.sems.allocated().values()))
        self.nc.all_engine_barrier()

    tile_mod.TileContext._drain_and_barrier = _drain_and_barrier
    tile_mod.TileContext._drain_split_patched = True


def _build_program():
    import concourse.bass as bass
    import concourse.mybir as mybir
    from concourse.tile import TileContext, add_dep_helper

    _patch_drain_split()

    import os
    fp32 = mybir.dt.float32
    Alu = mybir.AluOpType
    GSZ = KG * 7 * J                   # input floats per group per partition
    bulk_eng_name = os.environ.get("CCN_BULK_ENGINE", "gpsimd")

    nc = bass.Bass("TRN2")
    IN = nc.dram_tensor("inx", [PPART, WPL + T * 7 * J], fp32,
                        kind="ExternalInput").ap()
    OD = nc.dram_tensor("oout", [PPART, T, 3, J], fp32,
                        kind="ExternalOutput").ap()

    with TileContext(nc) as tc:
        with (
            tc.tile_pool(name="const", bufs=1) as cpool,
            tc.tile_pool(name="inp", bufs=2) as ipool,
            tc.tile_pool(name="blk", bufs=1) as bpool,
            tc.tile_pool(name="hist", bufs=2) as hpool,
            tc.tile_pool(name="tiny", bufs=5) as tpool,
        ):
            wts = cpool.tile([PPART, 12, J], fp32)
            s0c = cpool.tile([PPART, 4, J], fp32)
            dummy = cpool.tile([PPART, 1], fp32)

            bulk = nc.gpsimd if bulk_eng_name == "gpsimd" else nc.vector
            it_tiles = {}
            s_last = {}     # ci -> last DVE instruction reading g/h of chunk ci
            prev_y = None
            prev_pv = None
            w_pv_f0 = w_pv_f1 = w_ffp = w_fbp = w_pl = w_lt = None
            oh = None

            for ci in range(NCHUNK):
                gidx, half = divmod(ci, 2)
                if ci >= 2 and bulk_eng_name == "gpsimd":
                    # Data-free Pool absorber: waits for the DVE steps of
                    # chunk ci-2 (covers all DVE readers of the recycled
                    # input slot and of this chunk's G/H areas), so the DMA
                    # below only carries the Pool-readers wait and the bulk
                    # g/h writes carry only their Pool chain.
                    pobs = tpool.tile([PPART, 1], fp32, tag="pobs")
                    pb = nc.gpsimd.tensor_copy(out=pobs[:], in_=dummy[:])
                    add_dep_helper(s_last[ci - 2], pb.ins,
                                   reason="absorb DVE ticks on Pool")
                if half == 0:
                    itg = ipool.tile([PPART, WPL + GSZ], fp32, tag="it")
                    it_tiles[gidx] = itg
                    if gidx == 0:
                        nc.gpsimd.dma_start(out=itg[:], in_=IN[:, 0:WPL + GSZ])
                    else:
                        off = WPL + gidx * GSZ
                        nc.gpsimd.dma_start(out=itg[:, 0:GSZ],
                                            in_=IN[:, off:off + GSZ])
                else:
                    itg = it_tiles[gidx]

                boff = (WPL if gidx == 0 else 0) + half * K * 7 * J
                itv = itg[:, boff:boff + K * 7 * J].rearrange(
                    "p (k c j) -> p k c j", k=K, c=7, j=J
                )

                if ci == 0:
                    # Unpack weights/init-state (DVE observes the group-0 DMA
                    # here; later DVE consumers of wts/s0c are same-engine).
                    nc.vector.tensor_copy(
                        out=wts[:],
                        in_=itg[:, 0:12 * J].rearrange("p (c j) -> p c j",
                                                       c=12, j=J),
                    )
                    nc.vector.tensor_copy(
                        out=s0c[:],
                        in_=itg[:, 12 * J:16 * J].rearrange(
                            "p (c j) -> p c j", c=4, j=J),
                    )
                    w_pv_f0 = wts[:, 0:2, :]
                    w_pv_f1 = wts[:, 2:4, :]
                    w_ffp = wts[:, 4:6, :]
                    w_fbp = wts[:, 6:8, :]
                    w_pl = wts[:, 8:10, :]
                    w_lt = wts[:, 10:12, :]
                    nc.vector.tensor_copy(out=dummy[:], in_=wts[:, 0, 0:1])
                    # state init: A = a0/0.02, L = w_lat . pv0 (rotating
                    # state tiles: every step writes a fresh slot so the
                    # serial chain carries only RAW deps, which need no
                    # same-engine semaphore waits)
                    A_cur = tpool.tile([PPART, J], fp32, tag="A")
                    nc.vector.tensor_scalar(
                        out=A_cur[:], in0=s0c[:, 3, :], scalar1=50.0,
                        scalar2=None, op0=Alu.mult,
                    )
                    wi = tpool.tile([PPART, 2, J], fp32, tag="m")
                    nc.vector.tensor_tensor(
                        out=wi[:], in0=s0c[:, 1:3, :], in1=w_lt, op=Alu.mult
                    )
                    L_cur = tpool.tile([PPART, J], fp32, tag="L")
                    nc.vector.tensor_tensor(
                        out=L_cur[:], in0=wi[:, 0, :], in1=wi[:, 1, :],
                        op=Alu.add,
                    )
                    prev_y = s0c[:, 0, :]
                    prev_pv = s0c[:, 1:3, :]
                    if bulk_eng_name == "gpsimd":
                        # Pool absorber for the DVE weights-copy tick, so the
                        # first bulk op only waits on the group-0 DMA.
                        pobs0 = tpool.tile([PPART, 1], fp32, tag="pobs")
                        nc.gpsimd.tensor_copy(out=pobs0[:], in_=wts[:, 0, 0:1])

                # ---- bulk G/H on GPSIMD, in place into the input tile ----
                u = bpool.tile([PPART, K, 2, J], fp32, tag="u")
                v = bpool.tile([PPART, K, 2, J], fp32, tag="v")
                bshape = [PPART, K, 2, J]
                x0b = itv[:, :, 0:1, :].to_broadcast(bshape)
                x1b = itv[:, :, 1:2, :].to_broadcast(bshape)
                bulk.tensor_tensor(
                    out=u[:], in0=x0b,
                    in1=w_pv_f0[:, None, :, :].to_broadcast(bshape),
                    op=Alu.mult,
                )
                bulk.tensor_tensor(
                    out=v[:], in0=x1b,
                    in1=w_pv_f1[:, None, :, :].to_broadcast(bshape),
                    op=Alu.mult,
                )
                bulk.tensor_tensor(out=u[:], in0=u[:], in1=v[:],
                                    op=Alu.add)
                # last reads of x0/x1 (H ff products), then G overwrites 0:2
                bulk.tensor_tensor(
                    out=v[:], in0=itv[:, :, 0:2, :],
                    in1=w_ffp[:, None, :, :].to_broadcast(bshape), op=Alu.mult,
                )
                bulk.tensor_tensor(
                    out=itv[:, :, 0:2, :], in0=u[:], in1=itv[:, :, 4:6, :],
                    op=Alu.add,
                )
                bulk.tensor_tensor(
                    out=u[:], in0=itv[:, :, 2:4, :],
                    in1=w_fbp[:, None, :, :].to_broadcast(bshape), op=Alu.mult,
                )
                bulk.tensor_tensor(out=u[:], in0=u[:], in1=v[:],
                                    op=Alu.add)
                bulk.tensor_tensor(
                    out=v[:, :, 0, :], in0=u[:, :, 0, :], in1=u[:, :, 1, :],
                    op=Alu.add,
                )
                bulk.tensor_tensor(
                    out=itv[:, :, 6, :], in0=v[:, :, 0, :],
                    in1=itv[:, :, 6, :], op=Alu.add,
                )
                gv = itv[:, :, 0:2, :]
                hv = itv[:, :, 6, :]

                # ---- output tile (one per group) ----
                if half == 0:
                    oh = hpool.tile([PPART, KG, 3, J], fp32, tag="oh")
                    if gidx >= 2:
                        # absorb WAR-vs-out-DMA wait on the recycled slot
                        nc.vector.tensor_copy(out=oh[:, 0, 0, 0:1],
                                              in_=dummy[:])
                base = half * K

                # DVE observers absorb the Pool g/h-ready waits. The first
                # reads a DMA-only region (comp 2 = c0, never overwritten by
                # Pool) so the group-DMA tick and the Pool g-write tick land
                # on separate instructions.
                if half == 0 and gidx >= 1:
                    obs_d = tpool.tile([PPART, 1], fp32, tag="obs_d")
                    nc.vector.tensor_copy(out=obs_d[:], in_=itv[:, 0, 2, 0:1])
                obs_g = tpool.tile([PPART, 1], fp32, tag="obs_g")
                obs_h = tpool.tile([PPART, 1], fp32, tag="obs_h")
                nc.vector.tensor_copy(out=obs_g[:], in_=gv[:, 0, 0, 0:1])
                nc.vector.tensor_copy(out=obs_h[:], in_=hv[:, 0, 0:1])

                # ---- sequential per-step scan on DVE ----
                s_op = None
                for k in range(K):
                    G_t = gv[:, k, :, :]
                    H_t = hv[:, k, :]
                    A_new = tpool.tile([PPART, J], fp32, tag="A")
                    nc.vector.scalar_tensor_tensor(
                        out=A_new[:], in0=A_cur[:], scalar=0.98, in1=prev_y,
                        op0=Alu.mult, op1=Alu.add,
                    )
                    A_cur = A_new
                    p_t = tpool.tile([PPART, J], fp32, tag="p")
                    nc.vector.scalar_tensor_tensor(
                        out=p_t[:], in0=A_cur[:], scalar=-0.02, in1=H_t,
                        op0=Alu.mult, op1=Alu.add,
                    )
                    m_t = tpool.tile([PPART, 2, J], fp32, tag="m")
                    nc.vector.tensor_tensor(
                        out=m_t[:],
                        in0=prev_y[:, None, :].to_broadcast([PPART, 2, J]),
                        in1=w_pl, op=Alu.mult,
                    )
                    s_t = tpool.tile([PPART, 2, J], fp32, tag="s")
                    s_op = nc.vector.tensor_tensor(
                        out=s_t[:], in0=m_t[:], in1=G_t, op=Alu.add
                    )
                    qs = tpool.tile([PPART, 2, J], fp32, tag="qs")
                    nc.vector.tensor_scalar(
                        out=qs[:], in0=s_t[:], scalar1=0.0, scalar2=0.25,
                        op0=Alu.max, op1=Alu.mult,
                    )
                    nc.vector.scalar_tensor_tensor(
                        out=oh[:, base + k, 1:3, :], in0=prev_pv, scalar=0.75,
                        in1=qs[:], op0=Alu.mult, op1=Alu.add,
                    )
                    w_t = tpool.tile([PPART, 2, J], fp32, tag="w")
                    nc.vector.tensor_tensor(
                        out=w_t[:], in0=qs[:], in1=w_lt, op=Alu.mult
                    )
                    u1 = tpool.tile([PPART, J], fp32, tag="u1")
                    nc.vector.tensor_tensor(
                        out=u1[:], in0=w_t[:, 0, :], in1=w_t[:, 1, :],
                        op=Alu.add,
                    )
                    L_new = tpool.tile([PPART, J], fp32, tag="L")
                    nc.vector.scalar_tensor_tensor(
                        out=L_new[:], in0=L_cur[:], scalar=0.75, in1=u1[:],
                        op0=Alu.mult, op1=Alu.add,
                    )
                    L_cur = L_new
                    arg = tpool.tile([PPART, J], fp32, tag="arg")
                    nc.vector.tensor_tensor(
                        out=arg[:], in0=p_t[:], in1=L_cur[:], op=Alu.subtract
                    )
                    r_t = tpool.tile([PPART, J], fp32, tag="r")
                    nc.vector.tensor_scalar(
                        out=r_t[:], in0=arg[:], scalar1=0.1, scalar2=0.0,
                        op0=Alu.mult, op1=Alu.max,
                    )
                    nc.vector.scalar_tensor_tensor(
                        out=oh[:, base + k, 0, :], in0=prev_y, scalar=0.9,
                        in1=r_t[:], op0=Alu.mult, op1=Alu.add,
                    )
                    prev_y = oh[:, base + k, 0, :]
                    prev_pv = oh[:, base + k, 1:3, :]

                s_last[ci] = s_op.ins

                if half == 1:
                    nc.sync.dma_start(
                        out=OD[:, gidx * KG:(gidx + 1) * KG, :, :], in_=oh[:]
                    )

    return nc


def _get_program():
    if "nc" not in _PROGRAM_CACHE:
        _PROGRAM_CACHE["nc"] = _build_program()
    return _PROGRAM_CACHE["nc"]


def _to_pj(arr_tb):
    """[T, BS] (core slice) -> [PPART, T, J] with b = j*128 + p."""
    t = arr_tb.shape[0]
    return np.ascontiguousarray(arr_tb.reshape(t, J, PPART).transpose(2, 0, 1))


def _w_to_pj(arr_b):
    """[BS] -> [PPART, J]."""
    return np.ascontiguousarray(arr_b.reshape(J, PPART).T)


def kernel(**inputs):
    x = np.asarray(inputs["x"], np.float32)
    c = np.asarray(inputs["c"], np.float32)
    noise_p = np.asarray(inputs["noise_p"], np.float32)
    noise_y = np.asarray(inputs["noise_y"], np.float32)
    w_ff = np.asarray(inputs["w_ff"], np.float32)
    w_fb = np.asarray(inputs["w_fb"], np.float32)
    w_lat = np.asarray(inputs["w_lat"], np.float32)
    w_pv_lat = np.asarray(inputs["w_pv_lat"], np.float32)
    W_pv = np.asarray(inputs["W_pv"], np.float32)
    rc = np.asarray(inputs["receives_context"], np.float32)
    pv0 = np.asarray(inputs["pv0"], np.float32)
    y0 = np.asarray(inputs["y0"], np.float32)
    a0 = np.asarray(inputs["a0"], np.float32)

    w_fb_eff = w_fb * rc[None, :]

    in_maps = []
    for core in range(NCORES):
        lo, hi = core * BS, (core + 1) * BS
        inx = np.empty((PPART, WPL + T * 7 * J), np.float32)
        wt = inx[:, :WPL].reshape(PPART, 16, J)
        comp = inx[:, WPL:].reshape(PPART, T, 7, J)
        comp[:, :, 0, :] = _to_pj(x[:, lo:hi, 0])
        comp[:, :, 1, :] = _to_pj(x[:, lo:hi, 1])
        comp[:, :, 2, :] = _to_pj(c[:, lo:hi, 0])
        comp[:, :, 3, :] = _to_pj(c[:, lo:hi, 1])
        comp[:, :, 4, :] = _to_pj(noise_p[:, lo:hi, 0])
        comp[:, :, 5, :] = _to_pj(noise_p[:, lo:hi, 1])
        comp[:, :, 6, :] = _to_pj(noise_y[:, lo:hi])

        wt[:, 0, :] = _w_to_pj(W_pv[lo:hi, 0, 0])
        wt[:, 1, :] = _w_to_pj(W_pv[lo:hi, 1, 0])
        wt[:, 2, :] = _w_to_pj(W_pv[lo:hi, 0, 1])
        wt[:, 3, :] = _w_to_pj(W_pv[lo:hi, 1, 1])
        wt[:, 4, :] = _w_to_pj(w_ff[lo:hi, 0])
        wt[:, 5, :] = _w_to_pj(w_ff[lo:hi, 1])
        wt[:, 6, :] = _w_to_pj(w_fb_eff[lo:hi, 0])
        wt[:, 7, :] = _w_to_pj(w_fb_eff[lo:hi, 1])
        wt[:, 8, :] = _w_to_pj(w_pv_lat[lo:hi, 0])
        wt[:, 9, :] = _w_to_pj(w_pv_lat[lo:hi, 1])
        wt[:, 10, :] = _w_to_pj(w_lat[lo:hi, 0])
        wt[:, 11, :] = _w_to_pj(w_lat[lo:hi, 1])
        wt[:, 12, :] = _w_to_pj(y0[lo:hi])
        wt[:, 13, :] = _w_to_pj(pv0[lo:hi, 0])
        wt[:, 14, :] = _w_to_pj(pv0[lo:hi, 1])
        wt[:, 15, :] = _w_to_pj(a0[lo:hi])

        in_maps.append({"inx": inx})

    from concourse.bass_utils import run_bass_kernel_spmd

    nc = _get_program()
    res = run_bass_kernel_spmd(nc, in_maps, core_ids=list(range(NCORES)))
    _PROGRAM_CACHE["last_results"] = res

    out = np.empty((T, B, 4), np.float32)
    for core in range(NCORES):
        lo, hi = core * BS, (core + 1) * BS
        od = res.results[core]["oout"]          # [PPART, T, 3, J]
        y_next = od[:, :, 0, :].transpose(1, 2, 0).reshape(T, BS)
        pv = od[:, :, 1:3, :].transpose(1, 3, 0, 2).reshape(T, BS, 2)
        out[:, lo:hi, 1] = y_next
        out[0, lo:hi, 0] = y0[lo:hi]
        out[1:, lo:hi, 0] = y_next[:-1]
        out[:, lo:hi, 2:4] = pv
    return out

